# revision 22
# baseline (speedup 1.0000x reference)
"""Trainium2 Bass kernel for the nn_LSTMCell problem.

Strategy: data-parallel over the batch dim (4096 -> 8 cores x 512), weights
replicated. All on-chip compute happens in "transposed" orientation
(hidden on PSUM partitions, batch on the free dim) so every matmul operand
can be DMA'd in its natural, contiguous layout:

    gate.T[h, b] = sum_k W.T[k, h] * act.T[k, b]
    matmul(out[M=h128, N=b512], lhsT=WT_tile[K=k128, M=h128], rhs=actT[K=k128, N=b512])

Mixed precision: most gate matmuls run in fp8(e4m3) with DoubleRow perf mode
(2 k-tiles per matmul, ~2x PE throughput); the error-critical matrices (the
tanh'd cell-candidate gate, and optionally parts of the output gate) stay in
bf16. All operands are pre-scaled on the host (W*256, act*16) so fp8 values
sit in the normal range; the 2^-12 descale is folded into the gate activation
instruction. PSUM accumulation is fp32 throughout, as is all elementwise math.

Per matrix, CONFIG[name] = number of 128-wide k-tiles (out of 16) computed in
fp8-DoubleRow; the first 16-n8 k-tiles run in bf16. Both forms accumulate
into the same PSUM (uniform operand scaling makes that legal).

Per core:
  phase 1: for each of 16 h-tiles: i/f/g gate matmuls, sigmoid/tanh,
           c1 = f*c0 + i*tanh(g)  -> c1 (fp32, kept in SBUF + DMA'd out),
           c1 scaled+cast to fp8 (matmul operand for the o gate).
  phase 2: for each of 16 h-tiles: o gate matmuls (incl. W_co @ c1.T),
           o = sigmoid(...), h1 = o * tanh(c1), DMA out.
"""

import numpy as np
import ml_dtypes
from contextlib import ExitStack

BF = ml_dtypes.bfloat16
F8 = ml_dtypes.float8_e4m3

N_CORES = 8
P = 128          # partition dim / k-tile size / m-tile size
BATCH = 4096
IN_DIM = 2048
HID = 2048
B = BATCH // N_CORES          # 512, batch per core = matmul free dim
NK = 2048 // P                # 16, k-tiles per weight matrix contraction
MT = HID // P                 # 16, output h-tiles

W_NAMES = ["ii", "hi", "if_", "hf", "cf", "ic", "hc", "io", "ho", "co"]

# k-tiles (of 16) per matrix computed in fp8-DoubleRow; rest in bf16.
# The g gate (ic/hc, goes through tanh into c1) dominates the fp8 error and
# stays bf16; io/ho are the next-largest contributors.
CONFIG = {
    "ii": 16, "hi": 16,
    "if_": 16, "hf": 16, "cf": 16,
    "ic": 0, "hc": 0,
    "io": 16, "ho": 16, "co": 16,
}

SW = 256.0   # host-side weight scale (all matrices, both dtypes)
SA = 16.0    # host-side activation scale (x/h/c0 and on-device c1)
INV_S = 1.0 / (SW * SA)

# which activation operand forms are needed on device
_X_MATS = ("ii", "if_", "ic", "io")
_H_MATS = ("hi", "hf", "hc", "ho")
NEED_X8 = any(CONFIG[n] > 0 for n in _X_MATS)
NEED_X16 = any(CONFIG[n] < NK for n in _X_MATS)
NEED_H8 = any(CONFIG[n] > 0 for n in _H_MATS)
NEED_H16 = any(CONFIG[n] < NK for n in _H_MATS)
NEED_C8 = CONFIG["cf"] > 0
NEED_C16 = CONFIG["cf"] < NK
NEED_C18 = CONFIG["co"] > 0
NEED_C116 = CONFIG["co"] < NK


def _build(p, nk, mt, b):
    import concourse.tile as tile
    from concourse import bacc, mybir

    bf16, f32 = mybir.dt.bfloat16, mybir.dt.float32
    f8 = mybir.dt.float8e4
    Sig = mybir.ActivationFunctionType.Sigmoid
    Tanh = mybir.ActivationFunctionType.Tanh
    Copy = mybir.ActivationFunctionType.Copy
    DR = mybir.MatmulPerfMode.DoubleRow

    nc = bacc.Bacc(
        "TRN2",
        target_bir_lowering=False,
        debug=False,
        num_devices=N_CORES,
    )

    def act_in(name, dt):
        return nc.dram_tensor(name, [p, nk, b], dt, kind="ExternalInput").ap()

    xT8 = act_in("xT8", f8) if NEED_X8 else None
    xT16 = act_in("xT16", bf16) if NEED_X16 else None
    hT8 = act_in("hT8", f8) if NEED_H8 else None
    hT16 = act_in("hT16", bf16) if NEED_H16 else None
    cT8 = act_in("cT8", f8) if NEED_C8 else None
    cT16 = act_in("cT16", bf16) if NEED_C16 else None
    c0T = nc.dram_tensor("c0T", [p, mt, b], f32, kind="ExternalInput").ap()
    bias = nc.dram_tensor("bias", [p, mt, 4], f32, kind="ExternalInput").ap()

    w8, w16 = {}, {}
    for n in W_NAMES:
        n8 = CONFIG[n]
        if n8 > 0:
            w8[n] = nc.dram_tensor(
                f"w8_{n}", [mt, p, n8, p], f8, kind="ExternalInput").ap()
        if n8 < nk:
            w16[n] = nc.dram_tensor(
                f"w16_{n}", [mt, p, nk - n8, p], bf16, kind="ExternalInput").ap()

    ogT = nc.dram_tensor("ogT", [p, mt, b], f32, kind="ExternalOutput").ap()
    h1T = nc.dram_tensor("h1T", [p, mt, b], f32, kind="ExternalOutput").ap()
    c1T = nc.dram_tensor("c1T", [p, mt, b], f32, kind="ExternalOutput").ap()

    with tile.TileContext(nc) as tc, ExitStack() as ctx:
        acts = ctx.enter_context(tc.tile_pool(name="acts", bufs=1))
        wpool = ctx.enter_context(tc.tile_pool(name="w", bufs=3))
        cpool = ctx.enter_context(tc.tile_pool(name="c0", bufs=2))
        tpool = ctx.enter_context(tc.tile_pool(name="temps", bufs=2))
        ppool = ctx.enter_context(tc.tile_pool(name="psum", bufs=8, space="PSUM"))

        # resident activation tensors. Loads go on the gpsimd/sync DMA issue
        # queues, split into chunks so the first matmuls — which only need the
        # first x chunks plus one weight slab — start early.
        CH = 4  # k-tiles per DMA chunk
        sb = {}
        loads = []
        # spread the activation preload over four DMA issue queues so the
        # early m-tiles (whose matmuls consume data as fast as it lands) are
        # not bottlenecked on a single queue's descriptor rate. x8 gets a
        # small leading chunk so the very first matmul can start early.
        for key, need, src, dt, eng, chunks in (
            ("x8", NEED_X8, xT8, f8, nc.gpsimd, (2, 2, 4, 4, 4)),
            ("x16", NEED_X16, xT16, bf16, nc.sync, (2, 2, 4, 4, 4)),
            ("h8", NEED_H8, hT8, f8, nc.gpsimd, (4, 4, 4, 4)),
            ("h16", NEED_H16, hT16, bf16, nc.sync, (4, 4, 4, 4)),
            ("c8", NEED_C8, cT8, f8, nc.gpsimd, (8, 8)),
            ("c16", NEED_C16, cT16, bf16, nc.sync, (8, 8)),
        ):
            if need:
                sb[key] = acts.tile([p, nk, b], dt, tag=key, name=key + "_sb")
                loads.append((src, sb[key], eng, chunks))
        bias_sb = acts.tile([p, mt, 4], f32, tag="bias")
        nc.scalar.dma_start(bias_sb[:], bias[:])
        for src, dst, eng, chunks in loads:
            c = 0
            for ci, ch in enumerate(chunks):
                # the first x8 chunks gate the very first matmul: issue them
                # on the hardware-DGE scalar queue, which starts transfers
                # ~1us before the software-DGE gpsimd queue
                e = nc.scalar if (dst is sb.get("x8") and ci < 2) else eng
                e.dma_start(dst[:, c:c + ch, :], src[:, c:c + ch, :])
                c += ch
        c1f_sb = acts.tile([p, mt, b], f32, tag="c1f")    # new cell state, fp32
        c18_sb = (acts.tile([p, mt, b], f8, tag="c18", name="c18_sb")
                  if NEED_C18 else None)
        c116_sb = (acts.tile([p, mt, b], bf16, tag="c116", name="c116_sb")
                   if NEED_C116 else None)

        def load_w(name, tag, m, chunks=1, eng=None):
            """Load this matrix's bf16 part and fp8 part; returns (t16, t8)."""
            t16 = t8 = None
            n8 = CONFIG[name]
            if n8 < nk:
                nkp = nk - n8
                t16 = wpool.tile([p, nkp, p], bf16, tag=tag + "b")
                step = max(1, nkp // chunks)
                for c in range(0, nkp, step):
                    (eng or nc.sync).dma_start(
                        t16[:, c:c + step], w16[name][m, :, c:c + step])
            if n8 > 0:
                t8 = wpool.tile([p, n8, p], f8, tag=tag + "a")
                step = max(1, n8 // chunks)
                for c in range(0, n8, step):
                    (eng or nc.sync).dma_start(
                        t8[:, c:c + step], w8[name][m, :, c:c + step])
            return t16, t8

        def accum(ps, name, wt, a16, a8, first, last):
            """Emit all matmuls for one matrix: bf16 k-tiles then fp8 pairs."""
            t16, t8 = wt
            n8 = CONFIG[name]
            nb = nk - n8
            total = nb + n8 // 2
            idx = 0
            for ko in range(nb):
                nc.tensor.matmul(
                    ps[:], lhsT=t16[:, ko], rhs=a16[:, ko],
                    start=(first and idx == 0),
                    stop=(last and idx == total - 1),
                )
                idx += 1
            for ko2 in range(0, n8, 2):
                nc.tensor.matmul(
                    ps[:], lhsT=t8[:, ko2:ko2 + 2],
                    rhs=a8[:, nb + ko2:nb + ko2 + 2],
                    start=(first and idx == 0),
                    stop=(last and idx == total - 1),
                    perf_mode=DR,
                )
                idx += 1

        # ---- phase 1: i/f/g gates + new cell state ----
        # x-term weights load (and matmul) first so the first m-tile's PE work
        # starts as soon as x chunks land, while h/c still stream in.
        p2_pre = {}
        for m in range(mt):
            # m=0/m=1 slab issues go on the otherwise-idle scalar engine
            # queue: the sync/gpsimd queues are saturated streaming the
            # activation preload during the ramp.
            if m == 0:
                x_eng, x_ch, hc_eng, hc_ch = nc.scalar, 2, nc.scalar, 1
            elif m in (1, 2):
                # m=2's slabs would otherwise queue on sync behind the 4MB
                # bf16 activation preload and arrive ~5us late
                x_eng, x_ch, hc_eng, hc_ch = nc.scalar, 1, nc.scalar, 1
            else:
                x_eng, x_ch, hc_eng, hc_ch = None, 1, None, 1
            w_ii = load_w("ii", "w0", m, chunks=x_ch, eng=x_eng)
            w_if = load_w("if_", "w2", m, chunks=x_ch, eng=x_eng)
            w_ic = load_w("ic", "w5", m, chunks=x_ch, eng=x_eng)
            w_hi = load_w("hi", "w1", m, chunks=hc_ch, eng=hc_eng)
            w_hf = load_w("hf", "w3", m, chunks=hc_ch, eng=hc_eng)
            w_hc = load_w("hc", "w6", m, chunks=hc_ch, eng=hc_eng)
            w_cf = load_w("cf", "w4", m, chunks=hc_ch, eng=hc_eng)
            if m == mt - 2:
                # prefetch the first two phase-2 weight slab sets on the
                # scalar queue so phase 2 starts without a DMA stall
                # (phase-2's own loads would otherwise only issue after the
                # last phase-1 m-tile retires).
                for pm in (0, 1):
                    p2_pre[pm] = (load_w("io", "u0", pm, eng=nc.scalar),
                                  load_w("ho", "u1", pm, eng=nc.scalar),
                                  load_w("co", "u2", pm, eng=nc.scalar))

            x16, x8 = sb.get("x16"), sb.get("x8")
            h16, h8 = sb.get("h16"), sb.get("h8")
            ps_i = ppool.tile([p, b], f32, tag="ps")
            ps_f = ppool.tile([p, b], f32, tag="ps")
            ps_g = ppool.tile([p, b], f32, tag="ps")
            accum(ps_i, "ii", w_ii, x16, x8, True, False)
            accum(ps_f, "if_", w_if, x16, x8, True, False)
            accum(ps_g, "ic", w_ic, x16, x8, True, False)
            accum(ps_i, "hi", w_hi, h16, h8, False, True)
            accum(ps_f, "hf", w_hf, h16, h8, False, False)
            accum(ps_g, "hc", w_hc, h16, h8, False, True)
            accum(ps_f, "cf", w_cf, sb.get("c16"), sb.get("c8"), False, True)

            i_act = tpool.tile([p, b], f32, tag="i_act")
            nc.scalar.activation(i_act[:], ps_i[:], Sig,
                                 bias=bias_sb[:, m, 0:1], scale=INV_S)
            f_act = tpool.tile([p, b], f32, tag="f_act")
            nc.scalar.activation(f_act[:], ps_f[:], Sig,
                                 bias=bias_sb[:, m, 1:2], scale=INV_S)
            g_act = tpool.tile([p, b], f32, tag="g_act")
            nc.scalar.activation(g_act[:], ps_g[:], Tanh,
                                 bias=bias_sb[:, m, 2:3], scale=INV_S)

            c0_t = cpool.tile([p, b], f32, tag="c0")
            nc.gpsimd.dma_start(c0_t[:], c0T[:, m, :])

            t1 = tpool.tile([p, b], f32, tag="t1")
            nc.vector.tensor_mul(t1[:], f_act[:], c0_t[:])
            nc.vector.tensor_mul(i_act[:], i_act[:], g_act[:])
            c1_m = c1f_sb[:, m, :]
            nc.vector.tensor_add(c1_m, t1[:], i_act[:])
            if NEED_C18:
                nc.scalar.activation(c18_sb[:, m, :], c1_m, Copy, scale=SA)
            if NEED_C116:
                nc.vector.tensor_scalar_mul(c116_sb[:, m, :], c1_m, SA)
            nc.sync.dma_start(c1T[:, m, :], c1_m)

        # ---- phase 2: o gate + h1 ----
        for m in range(mt):
            if m in p2_pre:
                w_io, w_ho, w_co = p2_pre[m]
            else:
                eng2 = nc.gpsimd if m % 2 else None
                w_io = load_w("io", "u0", m, eng=eng2)
                w_ho = load_w("ho", "u1", m, eng=eng2)
                w_co = load_w("co", "u2", m, eng=eng2)

            ps_o = ppool.tile([p, b], f32, tag="ps")
            accum(ps_o, "io", w_io, sb.get("x16"), sb.get("x8"), True, False)
            accum(ps_o, "ho", w_ho, sb.get("h16"), sb.get("h8"), False, False)
            accum(ps_o, "co", w_co, c116_sb, c18_sb, False, True)

            o_act = tpool.tile([p, b], f32, tag="o_act")
            nc.scalar.activation(o_act[:], ps_o[:], Sig,
                                 bias=bias_sb[:, m, 3:4], scale=INV_S)
            tc1 = tpool.tile([p, b], f32, tag="tc1")
            nc.scalar.activation(tc1[:], c1f_sb[:, m, :], Tanh)
            h1_t = tpool.tile([p, b], f32, tag="h1")
            nc.vector.tensor_mul(h1_t[:], o_act[:], tc1[:])

            nc.sync.dma_start(ogT[:, m, :], o_act[:])
            nc.sync.dma_start(h1T[:, m, :], h1_t[:])

    nc.compile()
    return nc


_NC = None


def _get_nc():
    global _NC
    if _NC is None:
        _NC = _build(P, NK, MT, B)
    return _NC


# ---------------- host-side packing ----------------

def _pack_actT(a, dtype, scale=1.0):
    """(b, d) -> (128, d//128, b) with [ki, ko, b] = a[b, ko*128+ki]."""
    b, d = a.shape
    at = np.ascontiguousarray(a.T.reshape(d // P, P, b).transpose(1, 0, 2))
    if scale != 1.0:
        at = np.clip(at * scale, -240.0, 240.0)
    return at.astype(dtype, copy=False)


def _pack_w(W, dtype, kt_lo, kt_hi, scale):
    """(H, K) -> (H//128, 128, kt, 128) with [mt, ki, ko, m] = W[mt*128+m, ko*128+ki],
    keeping only k-tiles [kt_lo, kt_hi)."""
    H, K = W.shape
    r = (W * scale).reshape(H // P, P, K // P, P).transpose(0, 3, 2, 1)
    return np.ascontiguousarray(r[:, :, kt_lo:kt_hi]).astype(dtype)


def _unpack_out(o):
    """(128, mt, b) [p, m, b] -> (b, mt*128)."""
    p, m, b = o.shape
    return np.ascontiguousarray(o.transpose(2, 1, 0).reshape(b, m * p))


def kernel(x, h0, c0,
           W_ii, b_ii, W_hi, b_hi, W_if_, b_if_, W_hf, b_hf, W_cf, b_cf,
           W_ic, b_ic, W_hc, b_hc, W_io, b_io, W_ho, b_ho, W_co, b_co,
           _trace=False):
    from concourse.bass_utils import run_bass_kernel_spmd

    nc = _get_nc()

    x = np.asarray(x, dtype=np.float32)
    h0 = np.asarray(h0, dtype=np.float32)
    c0 = np.asarray(c0, dtype=np.float32)
    Ws = {n: np.asarray(a, dtype=np.float32)
          for n, a in zip(W_NAMES, (W_ii, W_hi, W_if_, W_hf, W_cf,
                                    W_ic, W_hc, W_io, W_ho, W_co))}
    (b_ii, b_hi, b_if_, b_hf, b_cf, b_ic, b_hc, b_io, b_ho, b_co) = [
        np.asarray(a, dtype=np.float32)
        for a in (b_ii, b_hi, b_if_, b_hf, b_cf, b_ic, b_hc, b_io, b_ho, b_co)
    ]

    # combined per-gate biases, packed [p, mt, gate]
    bias = np.stack(
        [
            (b_ii + b_hi).reshape(MT, P).T,
            (b_if_ + b_hf + b_cf).reshape(MT, P).T,
            (b_ic + b_hc).reshape(MT, P).T,
            (b_io + b_ho + b_co).reshape(MT, P).T,
        ],
        axis=2,
    ).astype(np.float32)

    w_packed = {}
    for n, W in Ws.items():
        n8 = CONFIG[n]
        nb = NK - n8
        if n8 > 0:
            w_packed[f"w8_{n}"] = _pack_w(W, F8, nb, NK, SW)
        if nb > 0:
            w_packed[f"w16_{n}"] = _pack_w(W, BF, 0, nb, SW)

    in_maps = []
    for core in range(N_CORES):
        s = slice(core * B, (core + 1) * B)
        m = {
            "c0T": _pack_actT(c0[s], np.float32),
            "bias": bias,
        }
        if NEED_X8:
            m["xT8"] = _pack_actT(x[s], F8, SA)
        if NEED_X16:
            m["xT16"] = _pack_actT(x[s], BF, SA)
        if NEED_H8:
            m["hT8"] = _pack_actT(h0[s], F8, SA)
        if NEED_H16:
            m["hT16"] = _pack_actT(h0[s], BF, SA)
        if NEED_C8:
            m["cT8"] = _pack_actT(c0[s], F8, SA)
        if NEED_C16:
            m["cT16"] = _pack_actT(c0[s], BF, SA)
        m.update(w_packed)
        in_maps.append(m)

    res = run_bass_kernel_spmd(nc, in_maps, list(range(N_CORES)), trace=_trace)

    o_g = np.empty((BATCH, HID), np.float32)
    h1 = np.empty((BATCH, HID), np.float32)
    c1 = np.empty((BATCH, HID), np.float32)
    for core in range(N_CORES):
        s = slice(core * B, (core + 1) * B)
        o_g[s] = _unpack_out(res.results[core]["ogT"])
        h1[s] = _unpack_out(res.results[core]["h1T"])
        c1[s] = _unpack_out(res.results[core]["c1T"])
    out = (o_g, h1, c1)
    if _trace:
        return out, res
    return out
